# revision 45
# baseline (speedup 1.0000x reference)
"""Trainium2 Bass kernel for CausalSelfAttention (no causal mask in reference).

Problem shapes: x [B=2, T=2048, C=1024], H=16 heads, D=64 head dim.
  q/k/v = x @ W{q,k,v}.T ; att = softmax(q k^T / sqrt(D)) ; y = att v
  out = y @ Wp.T + bp

Sharding over 8 NeuronCores: 4 head-groups (4 heads = 256 dims each) x 2
batches.  Core (g, b) computes a partial output for x[b] restricted to head
group g; the host sums the 4 head-group partials per batch and adds bp.

v5 — hybrid fp8 DoubleRow scores + engine rebalance (134.3us, was
150.2us; rel_absmax 0.0174 of the 0.02 budget):
* QKV projections: hi/lo-COMPENSATED fp8 DoubleRow (x = x_hi + x_lo,
  16W = w_hi + w_lo; q = x_hi w_hi + x_hi w_lo + x_lo w_hi) — bf16-class
  accuracy at half the bf16 PE cost.  Weights are pre-scaled by 16 so the
  lo residuals stay in e4m3's normal range.
* Scores, hybrid by head-pair (the out-projection mixes heads, so fp8
  noise on half the heads enters the max-error metric at sqrt(1/2)):
  - hp=0: fp16 q/k, plain matmuls (error-free scores, 1 cyc/row);
  - hp=1: fp8e4 DoubleRow at 0.5 cyc/row with the DR ko slots carrying
    K_hi and K_lo (compensated K, zero extra matmuls) while q8 rides both
    slots via a stride-0 broadcast AP — S = (K_hi+K_lo)^T q8.  Only q's
    fp8 quantization adds error.  Cuts PE scores 131072 -> 98304 cycles.
* exp: ACT native Exp (fp8 out) for 88 tiles + 40 DVE tiles via the
  one-op Schraudolph trick: round(S*scale*8/ln2 + 55.62) written as int8
  IS the fp8e4 bit pattern of exp.  (GPSIMD cannot read PSUM on real
  TRN2 — BIR verifier — so it only runs DMAs and memsets.)
* PV: P in fp8e4 x V hi/lo-compensated DoubleRow, chains jj-grouped so
  the last unit can interleave PV with its own exp stream; ones columns
  in V_hi emit the softmax denominator on PSUM rows 64:128 (V_lo carries
  no aug columns and only accumulates rows 0:64).  The final V_hi/V_lo
  pair is swapped so the group-closing stop lands on a full-height pass.
* Output projection in f32r (same PE cost as bf16, fp32-accurate).
  Partials summed on host in fp32 (+bp); DMA'd out as bf16.
* Tile deps follow program order: every filler is emitted after the
  writes it reads (k/q before consumer units, all 16 vg before any pv,
  pv norms before op).
"""

import numpy as np
import ml_dtypes

import concourse.bass as bass
import concourse.tile as tile
from concourse import mybir
from concourse.bacc import Bacc
from concourse.bass_utils import run_bass_kernel_spmd

BF16 = mybir.dt.bfloat16
FP16 = mybir.dt.float16
F32 = mybir.dt.float32
F32R = mybir.dt.float32r
F8 = mybir.dt.float8e4
I8 = mybir.dt.int8
NP_BF16 = ml_dtypes.bfloat16
NP_F8 = mybir.dt.np(F8)

P = 128
C = 1024
H = 16
D = 64
N_CORES = 8
N_GROUPS = 4              # head groups (tensor parallel)
N_BATCH = 2               # data parallel over B
HL = H // N_GROUPS        # 4 local heads
DL = HL * D               # 256 local head dims
CHUNK = 512               # t-chunk width (one PSUM bank of fp32)

DR = mybir.MatmulPerfMode.DoubleRow
EXP = mybir.ActivationFunctionType.Exp

WSCALE = 16.0             # host pre-scale on Wq/Wk/Wv (see module docstring)
ESCALE = 0.125 / (WSCALE * WSCALE)
SCH_A = ESCALE * 8.0 / np.log(2.0)
SCH_B = 55.62

# exp slot -> engine per unit (unit = 2*ch + hp).  Default engine is
# ACT (native Exp); listed slots run the Schraudolph tensor_scalar on
# DVE instead.  More DVE slots = shorter ACT chain but more Schraudolph
# error; 40 slots measured 134284ns @ rel_absmax 0.0174.  (GPSIMD cannot
# read PSUM on real TRN2 — BIR verifier NCC_IBVF — so exp tiles can only
# run on ACT or DVE.)
DVE_SLOTS = (
    (2, 5, 8, 11, 14),
    (2, 5, 8, 11, 14),
    (2, 5, 8, 11, 14),
    (2, 5, 8, 11, 14),
    (2, 5, 8, 11, 14),
    (2, 5, 8, 11, 14),
    (2, 5, 8, 11, 14),
    (1, 4, 7, 10, 13),
)


def build_program(T: int = 2048) -> bass.Bass:
    KO = C // P            # 8 k-tiles over the C contraction
    KPAIR = KO // 2        # 4 DoubleRow k-pairs
    TT = T // P            # 16 s/t tiles of 128
    NCH = T // CHUNK       # 4 t-chunks
    KP = DL // P           # 2 k-tiles over local head dims (outproj)

    nc = Bacc()
    xhi_d = nc.declare_dram_parameter("xhi", [C, T], F8, isOutput=False)
    xlo_d = nc.declare_dram_parameter("xlo", [C, T], F8, isOutput=False)
    # qkv weights arrive host-packed in SBUF layout [P, KO*DL] (2KB rows:
    # 256B rows would pay the sub-512B DMA descriptor penalty)
    w_d = {
        n: nc.declare_dram_parameter(n, [P, KO * DL], F8, isOutput=False)
        for n in ("wqh", "wql", "wkh", "wkl", "wvh", "wvl")
    }
    wpT_d = nc.declare_dram_parameter("wpT", [P, KP * C], F32R, isOutput=False)
    out_d = nc.declare_dram_parameter("out", [T, C], BF16, isOutput=True)

    with tile.TileContext(nc) as tc:
        with (
            tc.tile_pool(name="const", bufs=1) as cp,
            tc.tile_pool(name="att_s", bufs=2, space="PSUM") as att_s,
            tc.tile_pool(name="accy", bufs=2, space="PSUM") as accy,
            tc.tile_pool(name="accps", bufs=2, space="PSUM") as accps,
            tc.tile_pool(name="expp", bufs=38) as exp_pool,
            tc.tile_pool(name="normp", bufs=6) as norm_pool,
            tc.tile_pool(name="outp", bufs=4) as out_pool,
        ):
            xhi_sb = cp.tile([P, KO, T], F8)
            xlo_sb = cp.tile([P, KO, T], F8)
            w_sb = {n: cp.tile([P, KO, DL], F8, name=n) for n in w_d}
            wpT_sb = cp.tile([P, KP, C], F32R)
            # hybrid scores: head-pair hp=0 keeps bf16 q/k (error-free
            # scores); hp=1 uses fp8 q + hi/lo-compensated k in the DR ko
            # slots.  The out-projection mixes all heads, so the fp8 error
            # only enters at sqrt(1/2) weight.
            QTb_sb = cp.tile([P, T], FP16)
            KTb_sb = cp.tile([P, T], FP16)
            QT_sb = cp.tile([P, T], F8)
            KT_sb = cp.tile([P, 2, T], F8)
            # per (s-tile, head): V_hi = 64 V columns + 64 ones columns
            # (denominator rows); V_lo = 64 V columns only
            Vhi_sb = cp.tile([P, TT, HL, P], F8)
            Vlo_sb = cp.tile([P, TT, HL, D], F8)
            YT_sb = cp.tile([P, KP, T], F32R)

            # dummy matmuls on a memset tile keep the PE busy through the
            # initial DMA wait so the clock ramp completes before the first
            # real projection chain
            warm_sb = cp.tile([P, CHUNK], BF16)
            nc.vector.memset(warm_sb, 0.0)
            for _w in range(6):
                ps_w = accps.tile([P, CHUNK], F32, tag="acc", name="ps_w")
                nc.tensor.matmul(
                    ps_w, lhsT=warm_sb[:, 0:P], rhs=warm_sb, start=True, stop=True
                )

            # DMAs ordered by first use.  One DMA per (tensor, chunk) for x
            # (4KB rows), one per weight tensor: stays over the 500ns
            # descriptor-gen floor.
            def w_dma(eng, name):
                eng.dma_start(
                    out=w_sb[name][:, :, :],
                    in_=w_d[name][:, :].rearrange("p (ko d) -> p ko d", d=DL),
                )

            xhi_r = xhi_d[:, :].rearrange("(ko p) t -> p ko t", p=P)
            xlo_r = xlo_d[:, :].rearrange("(ko p) t -> p ko t", p=P)

            def x_dma(eng, sb, r, ch):
                sl = slice(ch * CHUNK, (ch + 1) * CHUNK)
                eng.dma_start(out=sb[:, :, sl], in_=r[:, :, sl])

            # Pool queue: k weights, x-lo ch0, q weights (prep deps first)
            w_dma(nc.gpsimd, "wkh")
            w_dma(nc.gpsimd, "wkl")
            x_dma(nc.gpsimd, xlo_sb, xlo_r, 0)
            w_dma(nc.gpsimd, "wqh")
            w_dma(nc.gpsimd, "wql")
            # SP queue: x-hi chunks, v weights, wp
            x_dma(nc.sync, xhi_sb, xhi_r, 0)
            x_dma(nc.sync, xhi_sb, xhi_r, 1)
            w_dma(nc.sync, "wvh")
            w_dma(nc.sync, "wvl")
            x_dma(nc.sync, xhi_sb, xhi_r, 2)
            x_dma(nc.sync, xhi_sb, xhi_r, 3)
            nc.sync.dma_start(
                out=wpT_sb[:, :, :],
                in_=wpT_d[:, :].rearrange("p (kp n) -> p kp n", n=C),
            )

            # V_hi aug ones (denominator weights), split so the Pool engine
            # stream can interleave its first exp tiles
            nc.gpsimd.memset(Vhi_sb[:, 0:8, :, D : 2 * D], 1.0)
            x_dma(nc.gpsimd, xlo_sb, xlo_r, 1)
            nc.gpsimd.memset(Vhi_sb[:, 8:TT, :, D : 2 * D], 1.0)
            x_dma(nc.gpsimd, xlo_sb, xlo_r, 2)
            x_dma(nc.gpsimd, xlo_sb, xlo_r, 3)

            # ---------- emission helpers ----------
            CHAINS = {  # compensated product: hi*hi + hi*lo + lo*hi
                "q": [(xhi_sb, "wqh"), (xhi_sb, "wql"), (xlo_sb, "wqh")],
                "k": [(xhi_sb, "wkh"), (xhi_sb, "wkl"), (xlo_sb, "wkh")],
                "v": [(xhi_sb, "wvh"), (xhi_sb, "wvl"), (xlo_sb, "wvh")],
            }

            def emit_qk_chain(which, j, ch, ci, state, on_act):
                # one compensation chain (4 DoubleRow matmuls); chain 0
                # allocates the accumulator, chain 2 drains it
                if ci == 0:
                    state["ps"] = accps.tile([P, CHUNK], F32, tag="acc", name="ps")
                ps = state["ps"]
                xs, wn = CHAINS[which][ci]
                for kk in range(KPAIR):
                    nc.tensor.matmul(
                        ps,
                        lhsT=w_sb[wn][:, 2 * kk : 2 * kk + 2, j * P : (j + 1) * P],
                        rhs=xs[:, 2 * kk : 2 * kk + 2, ch * CHUNK : (ch + 1) * CHUNK],
                        start=(ci == 0 and kk == 0),
                        stop=(ci == 2 and kk == KPAIR - 1),
                        perf_mode=DR,
                    )
                if ci == 2:
                    sl = slice(ch * CHUNK, (ch + 1) * CHUNK)
                    if j == 0:
                        dst = (QTb_sb if which == "q" else KTb_sb)[:, sl]
                        if on_act:
                            nc.scalar.copy(out=dst, in_=ps)
                        else:
                            nc.vector.tensor_copy(out=dst, in_=ps)
                    elif which == "q":
                        dst = QT_sb[:, sl]
                        if on_act:
                            nc.scalar.copy(out=dst, in_=ps)
                        else:
                            nc.vector.tensor_copy(out=dst, in_=ps)
                    else:
                        hi = KT_sb[:, 0, sl]
                        lo = KT_sb[:, 1, sl]
                        if on_act:
                            nc.scalar.copy(out=hi, in_=ps)
                        else:
                            nc.vector.tensor_copy(out=hi, in_=ps)
                        nc.vector.tensor_tensor(
                            out=lo, in0=ps, in1=hi,
                            op=mybir.AluOpType.subtract,
                        )

            def emit_qk_group(which, j, ch, on_act=False):
                state = {}
                for ci in range(3):
                    emit_qk_chain(which, j, ch, ci, state, on_act)

            def emit_v_chain(m, ci, state):
                if ci == 0:
                    state["ps"] = accps.tile([P, CHUNK], F32, tag="acc", name="ps")
                ps = state["ps"]
                xs, wn = CHAINS["v"][ci]
                for kk in range(KPAIR):
                    nc.tensor.matmul(
                        ps[:, 0:DL],
                        lhsT=xs[:, 2 * kk : 2 * kk + 2, m * P : (m + 1) * P],
                        rhs=w_sb[wn][:, 2 * kk : 2 * kk + 2, :],
                        start=(ci == 0 and kk == 0),
                        stop=(ci == 2 and kk == KPAIR - 1),
                        perf_mode=DR,
                    )
                if ci == 2:
                    vin = ps[:, 0:DL].rearrange("p (h e) -> p h e", e=D)
                    nc.vector.tensor_copy(out=Vhi_sb[:, m, :, 0:D], in_=vin)
                    nc.vector.tensor_tensor(
                        out=Vlo_sb[:, m, :, :],
                        in0=vin,
                        in1=Vhi_sb[:, m, :, 0:D],
                        op=mybir.AluOpType.subtract,
                    )

            exps = {}  # (ch, hp) -> list of 8 E tiles [P, 2, 2*CHUNK] fp8

            def emit_sexp(ch, hp, fillers=()):
                # score+exp stream for one (chunk, head-pair) unit; fillers
                # are PE work closures sprinkled between s-tiles so the PE
                # queue never head-of-line-blocks the exp engines
                t0 = ch * CHUNK
                u = 2 * ch + hp
                lst = []
                exps[(ch, hp)] = lst  # grows as tiles are created
                nf = len(fillers)
                fi = 0
                dve_s = DVE_SLOTS[u]
                for s in range(TT):
                    ps_s = att_s.tile([P, 2 * CHUNK], F32, tag="s", name="ps_s")
                    for ha in range(2):
                        dsl = slice(ha * D, (ha + 1) * D)
                        if hp == 0:
                            nc.tensor.matmul(
                                ps_s[:, ha * CHUNK : (ha + 1) * CHUNK],
                                lhsT=KTb_sb[dsl, s * P : (s + 1) * P],
                                rhs=QTb_sb[dsl, t0 : t0 + CHUNK],
                                start=True,
                                stop=True,
                            )
                        else:
                            nc.tensor.matmul(
                                ps_s[:, ha * CHUNK : (ha + 1) * CHUNK],
                                lhsT=KT_sb[dsl, :, s * P : (s + 1) * P],
                                rhs=QT_sb[dsl, t0 : t0 + CHUNK]
                                .unsqueeze(1)
                                .broadcast_to((D, 2, CHUNK)),
                                start=True,
                                stop=True,
                                perf_mode=DR,
                            )
                    if s % 2 == 0:
                        E = exp_pool.tile([P, 2, 2 * CHUNK], F8, tag="e", name="E")
                        lst.append(E)
                    dst = lst[-1][:, s % 2, :]
                    if s in dve_s:
                        nc.vector.tensor_scalar(
                            out=dst.bitcast(I8),
                            in0=ps_s,
                            scalar1=SCH_A,
                            scalar2=SCH_B,
                            op0=mybir.AluOpType.mult,
                            op1=mybir.AluOpType.add,
                        )
                    else:
                        nc.scalar.activation(out=dst, in_=ps_s, func=EXP, scale=ESCALE)
                    while fi < nf and fi < (s + 1) * nf // TT:
                        fillers[fi]()
                        fi += 1
                while fi < nf:
                    fillers[fi]()
                    fi += 1

            JJ = TT // 2
            N_PVCH = 2 * JJ

            def emit_pv_chains(ch, hp, ha, lo, hi, state):
                # jj-grouped chains: (V_hi, jj) then (V_lo, jj).  V_hi is
                # full-height (V + denominator aug); V_lo accumulates only
                # rows 0:64.  First chain (V_hi, 0) carries start (zeroes
                # the whole bank), last (V_lo, JJ-1) carries stop; the
                # normalization drain follows the last chain.
                t0 = ch * CHUNK
                lst = exps[(ch, hp)]
                h = hp * 2 + ha
                if lo == 0:
                    state["ps_y"] = accy.tile([P, CHUNK], F32, tag="y", name="ps_y")
                ps_y = state["ps_y"]
                for i in range(lo, hi):
                    jj, is_lo = divmod(i, 2)
                    if jj == JJ - 1:
                        # last pair swapped so the stop chain is full-height
                        # V_hi — the denominator rows' last writer must be
                        # the group-closing instruction, else the norm reads
                        # mid-accumulation-group
                        is_lo = 1 - is_lo
                    rhs = lst[jj][:, :, ha * CHUNK : (ha + 1) * CHUNK]
                    if is_lo:
                        nc.tensor.matmul(
                            ps_y[0:D, :],
                            lhsT=Vlo_sb[:, 2 * jj : 2 * jj + 2, h, :],
                            rhs=rhs,
                            start=False,
                            stop=(i == N_PVCH - 1),
                            perf_mode=DR,
                        )
                    else:
                        nc.tensor.matmul(
                            ps_y,
                            lhsT=Vhi_sb[:, 2 * jj : 2 * jj + 2, h, :],
                            rhs=rhs,
                            start=(i == 0),
                            stop=(i == N_PVCH - 1),
                            perf_mode=DR,
                        )
                if hi == N_PVCH:
                    if ha == 1:
                        exps.pop((ch, hp))
                    recip = norm_pool.tile([D, CHUNK], F32, tag="r", name="recip")
                    nc.vector.reciprocal(out=recip, in_=ps_y[D : 2 * D, :])
                    nc.vector.tensor_mul(
                        out=YT_sb[ha * D : (ha + 1) * D, hp, t0 : t0 + CHUNK],
                        in0=ps_y[0:D, :],
                        in1=recip,
                    )

            PV_PARTS = 4

            def emit_pv_ha(ch, hp, ha):
                state = {}
                for p in range(PV_PARTS):
                    emit_pv_chains(ch, hp, ha, N_PVCH * p // PV_PARTS,
                                   N_PVCH * (p + 1) // PV_PARTS, state)

            def pv_split(ch, hp, ha, cut):
                """Closures for chains [0, cut) in steps of 4, plus a tail fn
                emitting [cut, N_PVCH) + drain; all share one accumulator."""
                state = {}
                fills = [
                    (lambda lo=lo: emit_pv_chains(
                        ch, hp, ha, lo, min(lo + 4, cut), state))
                    for lo in range(0, cut, 4)
                ]
                def tail():
                    emit_pv_chains(ch, hp, ha, cut, N_PVCH, state)
                return fills, tail

            def emit_outproj_step(ch, mt, n2, kk, state, last=False):
                m = ch * (CHUNK // P) + mt
                if kk == 0 and n2 == 0:
                    state["o_sb"] = out_pool.tile([P, C], BF16, tag="o", name="o_sb")
                if kk == 0:
                    state[n2] = accps.tile([P, CHUNK], F32, tag="acc", name="ps_o")
                ps_o = state[n2]
                nc.tensor.matmul(
                    ps_o,
                    lhsT=YT_sb[:, kk, m * P : (m + 1) * P],
                    rhs=wpT_sb[:, kk, n2 * CHUNK : (n2 + 1) * CHUNK],
                    start=(kk == 0),
                    stop=(kk == KP - 1),
                )
                if kk == KP - 1:
                    o_sb = state["o_sb"]
                    dst = o_sb[:, n2 * CHUNK : (n2 + 1) * CHUNK]
                    # in the tail the exp stream is done: put half the drain
                    # copies on the freed ACT
                    if last and n2 % 2 == 0:
                        nc.scalar.copy(out=dst, in_=ps_o)
                    else:
                        nc.vector.tensor_copy(out=dst, in_=ps_o)
                    if n2 == C // CHUNK - 1:
                        eng = nc.sync if m % 2 == 0 else nc.gpsimd
                        eng.dma_start(
                            out=out_d[m * P : (m + 1) * P, :],
                            in_=o_sb,
                        )

            def emit_outproj_m(ch, mt, last=False):
                state = {}
                for n2 in range(C // CHUNK):
                    for kk in range(KP):
                        emit_outproj_step(ch, mt, n2, kk, state, last)

            # ---------- emission order ----------
            # Prep: only chunk-0 K and Q (copies on the still-idle ACT) so
            # the score stream starts as soon as x(ch0) lands; every other
            # projection, PV and outproj rides the stream as fillers with
            # deadlines encoded by position.
            emit_qk_group("k", 0, 0, on_act=True)
            emit_qk_group("q", 0, 0, on_act=True)

            def qk(which, j, ch):
                state = {}
                return [
                    (lambda ci=ci: emit_qk_chain(which, j, ch, ci, state, False))
                    for ci in range(3)
                ]

            def vg(m):
                state = {}
                return [
                    (lambda ci=ci: emit_v_chain(m, ci, state))
                    for ci in range(3)
                ]

            def pv(ch, hp, ha):
                state = {}
                return [
                    (lambda pt=pt: emit_pv_chains(
                        ch, hp, ha, N_PVCH * pt // PV_PARTS,
                        N_PVCH * (pt + 1) // PV_PARTS, state))
                    for pt in range(PV_PARTS)
                ]

            def op(ch, mt):
                state = {}
                return [
                    (lambda n2=n2, kk=kk: emit_outproj_step(ch, mt, n2, kk, state))
                    for n2 in range(2) for kk in range(KP)
                ]

            def flat(*fillers):
                return [f for sub in fillers for f in sub]

            # NOTE: Tile dependencies follow program order — every filler
            # must be emitted AFTER the writes it reads (a pv group needs
            # ALL 16 vg drains; scores of unit (ch,hp) need q(hp,ch) and
            # every k(hp,*) emitted in earlier positions).
            emit_sexp(0, 0, flat(
                qk("k", 0, 1), qk("k", 0, 2), qk("k", 0, 3),
                qk("k", 1, 0), qk("q", 1, 0),
            ))
            emit_sexp(0, 1, flat(
                qk("k", 1, 1), qk("k", 1, 2), qk("k", 1, 3),
                qk("q", 0, 1), vg(0), vg(1), vg(2), vg(3), vg(4),
            ))
            emit_sexp(1, 0, flat(
                qk("q", 1, 1), vg(5), vg(6), vg(7), vg(8), vg(9),
                vg(10),
            ))
            emit_sexp(1, 1, flat(
                qk("q", 0, 2), vg(11), vg(12), vg(13), vg(14), vg(15),
                pv(0, 0, 0), pv(0, 0, 1),
            ))
            emit_sexp(2, 0, flat(
                qk("q", 1, 2), pv(0, 1, 0), pv(0, 1, 1), pv(1, 0, 0),
            ))
            emit_sexp(2, 1, flat(
                qk("q", 0, 3), pv(1, 0, 1), pv(1, 1, 0), pv(1, 1, 1),
                op(0, 0), op(0, 1), op(0, 2),
            ))
            emit_sexp(3, 0, flat(
                qk("q", 1, 3), pv(2, 0, 0), pv(2, 0, 1),
                op(0, 3), op(1, 0), op(1, 1), op(1, 2),
            ))
            pv310_fills, pv310_tail = pv_split(3, 1, 0, 12)
            pv311_fills, pv311_tail = pv_split(3, 1, 1, 12)
            emit_sexp(3, 1, flat(
                pv(2, 1, 0), pv(2, 1, 1), pv(3, 0, 0), pv(3, 0, 1),
                op(1, 3), op(2, 0), pv310_fills, pv311_fills,
            ))
            # tail: pv completions first (they gate op(3) via the norms),
            # then ch2's remaining out tiles fill the PE while norms drain
            pv310_tail()
            pv311_tail()
            emit_outproj_m(2, 1, last=True)
            emit_outproj_m(2, 2, last=True)
            emit_outproj_m(2, 3, last=True)
            for mt in range(CHUNK // P):
                emit_outproj_m(3, mt, last=True)
    nc.finalize()
    return nc


def shard_inputs(x, Wk, Wq, Wv, Wp, T=2048):
    """Build the 8 per-core input dicts (hi/lo fp8 splits, host scaling)."""
    x = np.asarray(x, np.float32)
    Wk = np.asarray(Wk, np.float32)
    Wq = np.asarray(Wq, np.float32)
    Wv = np.asarray(Wv, np.float32)
    Wp = np.asarray(Wp, np.float32)

    def split8(a):
        hi = a.astype(NP_F8)
        lo = (a - hi.astype(np.float32)).astype(NP_F8)
        return hi, lo

    def pack(w):  # [C, DL] -> SBUF layout [P, KO*DL]
        KO = C // P
        return np.ascontiguousarray(
            w.reshape(KO, P, DL).transpose(1, 0, 2).reshape(P, KO * DL)
        )

    xs = []
    for b in range(x.shape[0]):
        xT = np.ascontiguousarray(x[b, :T].T)
        hi, lo = split8(xT)
        xs.append((np.ascontiguousarray(hi), np.ascontiguousarray(lo)))

    in_maps = []
    for g in range(N_GROUPS):
        sl = slice(g * DL, (g + 1) * DL)
        m = {}
        for n, W in (("wq", Wq), ("wk", Wk), ("wv", Wv)):
            hi, lo = split8(np.ascontiguousarray(W[sl].T * WSCALE))
            m[n + "h"] = pack(hi)
            m[n + "l"] = pack(lo)
        wp = (Wp[:, sl].T / WSCALE).astype(np.float32)  # [DL, C]
        KP = DL // P
        m["wpT"] = np.ascontiguousarray(
            wp.reshape(KP, P, C).transpose(1, 0, 2).reshape(P, KP * C)
        )
        for b in range(len(xs)):
            im = dict(m)
            im["xhi"], im["xlo"] = xs[b]
            in_maps.append(im)
    return in_maps


_PROGRAM = None


def kernel(x, Wk, Wq, Wv, Wp, bp):
    global _PROGRAM
    x = np.asarray(x, np.float32)
    bp = np.asarray(bp, np.float32)
    B, T, _ = x.shape

    if _PROGRAM is None:
        _PROGRAM = build_program(T)
    nc = _PROGRAM

    in_maps = shard_inputs(x, Wk, Wq, Wv, Wp, T=T)
    res = run_bass_kernel_spmd(nc, in_maps, core_ids=list(range(N_CORES)))
    parts = [r["out"] for r in res.results]

    out = np.zeros((B, T, C), np.float32)
    for g in range(N_GROUPS):
        for b in range(B):
            out[b] += parts[g * N_BATCH + b].astype(np.float32)
    out += bp
    return out


# revision 48
# speedup vs baseline: 1.0013x; 1.0013x over previous
"""Trainium2 Bass kernel for CausalSelfAttention (no causal mask in reference).

Problem shapes: x [B=2, T=2048, C=1024], H=16 heads, D=64 head dim.
  q/k/v = x @ W{q,k,v}.T ; att = softmax(q k^T / sqrt(D)) ; y = att v
  out = y @ Wp.T + bp

Sharding over 8 NeuronCores: 4 head-groups (4 heads = 256 dims each) x 2
batches.  Core (g, b) computes a partial output for x[b] restricted to head
group g; the host sums the 4 head-group partials per batch and adds bp.

v5 — hybrid fp8 DoubleRow scores + engine rebalance (134.3us, was
150.2us; rel_absmax 0.0174 of the 0.02 budget):
* QKV projections: hi/lo-COMPENSATED fp8 DoubleRow (x = x_hi + x_lo,
  16W = w_hi + w_lo; q = x_hi w_hi + x_hi w_lo + x_lo w_hi) — bf16-class
  accuracy at half the bf16 PE cost.  Weights are pre-scaled by 16 so the
  lo residuals stay in e4m3's normal range.
* Scores, hybrid by head-pair (the out-projection mixes heads, so fp8
  noise on half the heads enters the max-error metric at sqrt(1/2)):
  - hp=0: fp16 q/k, plain matmuls (error-free scores, 1 cyc/row);
  - hp=1: fp8e4 DoubleRow at 0.5 cyc/row with the DR ko slots carrying
    K_hi and K_lo (compensated K, zero extra matmuls) while q8 rides both
    slots via a stride-0 broadcast AP — S = (K_hi+K_lo)^T q8.  Only q's
    fp8 quantization adds error.  Cuts PE scores 131072 -> 98304 cycles.
* exp: ACT native Exp (fp8 out) for 88 tiles + 40 DVE tiles via the
  one-op Schraudolph trick: round(S*scale*8/ln2 + 55.62) written as int8
  IS the fp8e4 bit pattern of exp.  (GPSIMD cannot read PSUM on real
  TRN2 — BIR verifier — so it only runs DMAs and memsets.)
* PV: P in fp8e4 x V hi/lo-compensated DoubleRow, chains jj-grouped so
  the last unit can interleave PV with its own exp stream; ones columns
  in V_hi emit the softmax denominator on PSUM rows 64:128 (V_lo carries
  no aug columns and only accumulates rows 0:64).  The final V_hi/V_lo
  pair is swapped so the group-closing stop lands on a full-height pass.
* Output projection in f32r (same PE cost as bf16, fp32-accurate).
  Partials summed on host in fp32 (+bp); DMA'd out as bf16.
* Tile deps follow program order: every filler is emitted after the
  writes it reads (k/q before consumer units, all 16 vg before any pv,
  pv norms before op).
"""

import numpy as np
import ml_dtypes

import concourse.bass as bass
import concourse.tile as tile
from concourse import mybir
from concourse.bacc import Bacc
from concourse.bass_utils import run_bass_kernel_spmd

BF16 = mybir.dt.bfloat16
FP16 = mybir.dt.float16
F32 = mybir.dt.float32
F32R = mybir.dt.float32r
F8 = mybir.dt.float8e4
I8 = mybir.dt.int8
NP_BF16 = ml_dtypes.bfloat16
NP_F8 = mybir.dt.np(F8)

P = 128
C = 1024
H = 16
D = 64
N_CORES = 8
N_GROUPS = 4              # head groups (tensor parallel)
N_BATCH = 2               # data parallel over B
HL = H // N_GROUPS        # 4 local heads
DL = HL * D               # 256 local head dims
CHUNK = 512               # t-chunk width (one PSUM bank of fp32)

DR = mybir.MatmulPerfMode.DoubleRow
EXP = mybir.ActivationFunctionType.Exp

WSCALE = 16.0             # host pre-scale on Wq/Wk/Wv (see module docstring)
ESCALE = 0.125 / (WSCALE * WSCALE)
SCH_A = ESCALE * 8.0 / np.log(2.0)
SCH_B = 55.62

# exp slot -> engine per unit (unit = 2*ch + hp).  Default engine is
# ACT (native Exp); listed slots run the Schraudolph tensor_scalar on
# DVE instead.  More DVE slots = shorter ACT chain but more Schraudolph
# error; 40 slots measured 134284ns @ rel_absmax 0.0174.  (GPSIMD cannot
# read PSUM on real TRN2 — BIR verifier NCC_IBVF — so exp tiles can only
# run on ACT or DVE.)
DVE_SLOTS = (
    (2, 5, 8, 11, 14),
    (2, 5, 8, 11, 14),
    (2, 5, 8, 11, 14),
    (2, 5, 8, 11, 14),
    (2, 5, 8, 11, 14),
    (2, 5, 8, 11, 14),
    (2, 5, 8, 11, 14),
    (1, 4, 7, 10, 12, 14),
)


def build_program(T: int = 2048) -> bass.Bass:
    KO = C // P            # 8 k-tiles over the C contraction
    KPAIR = KO // 2        # 4 DoubleRow k-pairs
    TT = T // P            # 16 s/t tiles of 128
    NCH = T // CHUNK       # 4 t-chunks
    KP = DL // P           # 2 k-tiles over local head dims (outproj)

    nc = Bacc()
    xhi_d = nc.declare_dram_parameter("xhi", [C, T], F8, isOutput=False)
    xlo_d = nc.declare_dram_parameter("xlo", [C, T], F8, isOutput=False)
    # qkv weights arrive host-packed in SBUF layout [P, KO*DL] (2KB rows:
    # 256B rows would pay the sub-512B DMA descriptor penalty)
    w_d = {
        n: nc.declare_dram_parameter(n, [P, KO * DL], F8, isOutput=False)
        for n in ("wqh", "wql", "wkh", "wkl", "wvh", "wvl")
    }
    wpT_d = nc.declare_dram_parameter("wpT", [P, KP * C], F32R, isOutput=False)
    out_d = nc.declare_dram_parameter("out", [T, C], BF16, isOutput=True)

    with tile.TileContext(nc) as tc:
        with (
            tc.tile_pool(name="const", bufs=1) as cp,
            tc.tile_pool(name="att_s", bufs=2, space="PSUM") as att_s,
            tc.tile_pool(name="accy", bufs=2, space="PSUM") as accy,
            tc.tile_pool(name="accps", bufs=2, space="PSUM") as accps,
            tc.tile_pool(name="expp", bufs=38) as exp_pool,
            tc.tile_pool(name="normp", bufs=6) as norm_pool,
            tc.tile_pool(name="outp", bufs=4) as out_pool,
        ):
            xhi_sb = cp.tile([P, KO, T], F8)
            xlo_sb = cp.tile([P, KO, T], F8)
            w_sb = {n: cp.tile([P, KO, DL], F8, name=n) for n in w_d}
            wpT_sb = cp.tile([P, KP, C], F32R)
            # hybrid scores: head-pair hp=0 keeps bf16 q/k (error-free
            # scores); hp=1 uses fp8 q + hi/lo-compensated k in the DR ko
            # slots.  The out-projection mixes all heads, so the fp8 error
            # only enters at sqrt(1/2) weight.
            QTb_sb = cp.tile([P, T], FP16)
            KTb_sb = cp.tile([P, T], FP16)
            QT_sb = cp.tile([P, T], F8)
            KT_sb = cp.tile([P, 2, T], F8)
            # per (s-tile, head): V_hi = 64 V columns + 64 ones columns
            # (denominator rows); V_lo = 64 V columns only
            Vhi_sb = cp.tile([P, TT, HL, P], F8)
            Vlo_sb = cp.tile([P, TT, HL, D], F8)
            YT_sb = cp.tile([P, KP, T], F32R)

            # dummy matmuls on a memset tile keep the PE busy through the
            # initial DMA wait so the clock ramp completes before the first
            # real projection chain
            warm_sb = cp.tile([P, CHUNK], BF16)
            nc.vector.memset(warm_sb, 0.0)
            for _w in range(6):
                ps_w = accps.tile([P, CHUNK], F32, tag="acc", name="ps_w")
                nc.tensor.matmul(
                    ps_w, lhsT=warm_sb[:, 0:P], rhs=warm_sb, start=True, stop=True
                )

            # DMAs ordered by first use.  One DMA per (tensor, chunk) for x
            # (4KB rows), one per weight tensor: stays over the 500ns
            # descriptor-gen floor.
            def w_dma(eng, name):
                eng.dma_start(
                    out=w_sb[name][:, :, :],
                    in_=w_d[name][:, :].rearrange("p (ko d) -> p ko d", d=DL),
                )

            xhi_r = xhi_d[:, :].rearrange("(ko p) t -> p ko t", p=P)
            xlo_r = xlo_d[:, :].rearrange("(ko p) t -> p ko t", p=P)

            def x_dma(eng, sb, r, ch):
                sl = slice(ch * CHUNK, (ch + 1) * CHUNK)
                eng.dma_start(out=sb[:, :, sl], in_=r[:, :, sl])

            # Pool queue: k weights, x-lo ch0, q weights (prep deps first)
            w_dma(nc.gpsimd, "wkh")
            w_dma(nc.gpsimd, "wkl")
            x_dma(nc.gpsimd, xlo_sb, xlo_r, 0)
            w_dma(nc.gpsimd, "wqh")
            w_dma(nc.gpsimd, "wql")
            # SP queue: x-hi chunks, v weights, wp
            x_dma(nc.sync, xhi_sb, xhi_r, 0)
            x_dma(nc.sync, xhi_sb, xhi_r, 1)
            w_dma(nc.sync, "wvh")
            w_dma(nc.sync, "wvl")
            x_dma(nc.sync, xhi_sb, xhi_r, 2)
            x_dma(nc.sync, xhi_sb, xhi_r, 3)
            nc.sync.dma_start(
                out=wpT_sb[:, :, :],
                in_=wpT_d[:, :].rearrange("p (kp n) -> p kp n", n=C),
            )

            # V_hi aug ones (denominator weights), split so the Pool engine
            # stream can interleave its first exp tiles
            nc.gpsimd.memset(Vhi_sb[:, 0:8, :, D : 2 * D], 1.0)
            x_dma(nc.gpsimd, xlo_sb, xlo_r, 1)
            nc.gpsimd.memset(Vhi_sb[:, 8:TT, :, D : 2 * D], 1.0)
            x_dma(nc.gpsimd, xlo_sb, xlo_r, 2)
            x_dma(nc.gpsimd, xlo_sb, xlo_r, 3)

            # ---------- emission helpers ----------
            CHAINS = {  # compensated product: hi*hi + hi*lo + lo*hi
                "q": [(xhi_sb, "wqh"), (xhi_sb, "wql"), (xlo_sb, "wqh")],
                "k": [(xhi_sb, "wkh"), (xhi_sb, "wkl"), (xlo_sb, "wkh")],
                "v": [(xhi_sb, "wvh"), (xhi_sb, "wvl"), (xlo_sb, "wvh")],
            }

            def emit_qk_chain(which, j, ch, ci, state, on_act):
                # one compensation chain (4 DoubleRow matmuls); chain 0
                # allocates the accumulator, chain 2 drains it
                if ci == 0:
                    state["ps"] = accps.tile([P, CHUNK], F32, tag="acc", name="ps")
                ps = state["ps"]
                xs, wn = CHAINS[which][ci]
                for kk in range(KPAIR):
                    nc.tensor.matmul(
                        ps,
                        lhsT=w_sb[wn][:, 2 * kk : 2 * kk + 2, j * P : (j + 1) * P],
                        rhs=xs[:, 2 * kk : 2 * kk + 2, ch * CHUNK : (ch + 1) * CHUNK],
                        start=(ci == 0 and kk == 0),
                        stop=(ci == 2 and kk == KPAIR - 1),
                        perf_mode=DR,
                    )
                if ci == 2:
                    sl = slice(ch * CHUNK, (ch + 1) * CHUNK)
                    if j == 0:
                        dst = (QTb_sb if which == "q" else KTb_sb)[:, sl]
                        if on_act:
                            nc.scalar.copy(out=dst, in_=ps)
                        else:
                            nc.vector.tensor_copy(out=dst, in_=ps)
                    elif which == "q":
                        dst = QT_sb[:, sl]
                        if on_act:
                            nc.scalar.copy(out=dst, in_=ps)
                        else:
                            nc.vector.tensor_copy(out=dst, in_=ps)
                    else:
                        hi = KT_sb[:, 0, sl]
                        lo = KT_sb[:, 1, sl]
                        if on_act:
                            nc.scalar.copy(out=hi, in_=ps)
                        else:
                            nc.vector.tensor_copy(out=hi, in_=ps)
                        nc.vector.tensor_tensor(
                            out=lo, in0=ps, in1=hi,
                            op=mybir.AluOpType.subtract,
                        )

            def emit_qk_group(which, j, ch, on_act=False):
                state = {}
                for ci in range(3):
                    emit_qk_chain(which, j, ch, ci, state, on_act)

            def emit_v_chain(m, ci, state):
                if ci == 0:
                    state["ps"] = accps.tile([P, CHUNK], F32, tag="acc", name="ps")
                ps = state["ps"]
                xs, wn = CHAINS["v"][ci]
                for kk in range(KPAIR):
                    nc.tensor.matmul(
                        ps[:, 0:DL],
                        lhsT=xs[:, 2 * kk : 2 * kk + 2, m * P : (m + 1) * P],
                        rhs=w_sb[wn][:, 2 * kk : 2 * kk + 2, :],
                        start=(ci == 0 and kk == 0),
                        stop=(ci == 2 and kk == KPAIR - 1),
                        perf_mode=DR,
                    )
                if ci == 2:
                    vin = ps[:, 0:DL].rearrange("p (h e) -> p h e", e=D)
                    nc.vector.tensor_copy(out=Vhi_sb[:, m, :, 0:D], in_=vin)
                    nc.vector.tensor_tensor(
                        out=Vlo_sb[:, m, :, :],
                        in0=vin,
                        in1=Vhi_sb[:, m, :, 0:D],
                        op=mybir.AluOpType.subtract,
                    )

            exps = {}  # (ch, hp) -> list of 8 E tiles [P, 2, 2*CHUNK] fp8

            def emit_sexp(ch, hp, fillers=()):
                # score+exp stream for one (chunk, head-pair) unit; fillers
                # are PE work closures sprinkled between s-tiles so the PE
                # queue never head-of-line-blocks the exp engines
                t0 = ch * CHUNK
                u = 2 * ch + hp
                lst = []
                exps[(ch, hp)] = lst  # grows as tiles are created
                nf = len(fillers)
                fi = 0
                dve_s = DVE_SLOTS[u]
                for s in range(TT):
                    ps_s = att_s.tile([P, 2 * CHUNK], F32, tag="s", name="ps_s")
                    for ha in range(2):
                        dsl = slice(ha * D, (ha + 1) * D)
                        if hp == 0:
                            nc.tensor.matmul(
                                ps_s[:, ha * CHUNK : (ha + 1) * CHUNK],
                                lhsT=KTb_sb[dsl, s * P : (s + 1) * P],
                                rhs=QTb_sb[dsl, t0 : t0 + CHUNK],
                                start=True,
                                stop=True,
                            )
                        else:
                            nc.tensor.matmul(
                                ps_s[:, ha * CHUNK : (ha + 1) * CHUNK],
                                lhsT=KT_sb[dsl, :, s * P : (s + 1) * P],
                                rhs=QT_sb[dsl, t0 : t0 + CHUNK]
                                .unsqueeze(1)
                                .broadcast_to((D, 2, CHUNK)),
                                start=True,
                                stop=True,
                                perf_mode=DR,
                            )
                    if s % 2 == 0:
                        E = exp_pool.tile([P, 2, 2 * CHUNK], F8, tag="e", name="E")
                        lst.append(E)
                    dst = lst[-1][:, s % 2, :]
                    if s in dve_s:
                        nc.vector.tensor_scalar(
                            out=dst.bitcast(I8),
                            in0=ps_s,
                            scalar1=SCH_A,
                            scalar2=SCH_B,
                            op0=mybir.AluOpType.mult,
                            op1=mybir.AluOpType.add,
                        )
                    else:
                        nc.scalar.activation(out=dst, in_=ps_s, func=EXP, scale=ESCALE)
                    while fi < nf and fi < (s + 1) * nf // TT:
                        fillers[fi]()
                        fi += 1
                while fi < nf:
                    fillers[fi]()
                    fi += 1

            JJ = TT // 2
            N_PVCH = 2 * JJ

            def emit_pv_chains(ch, hp, ha, lo, hi, state):
                # jj-grouped chains: (V_hi, jj) then (V_lo, jj).  V_hi is
                # full-height (V + denominator aug); V_lo accumulates only
                # rows 0:64.  First chain (V_hi, 0) carries start (zeroes
                # the whole bank), last (V_lo, JJ-1) carries stop; the
                # normalization drain follows the last chain.
                t0 = ch * CHUNK
                lst = exps[(ch, hp)]
                h = hp * 2 + ha
                if lo == 0:
                    state["ps_y"] = accy.tile([P, CHUNK], F32, tag="y", name="ps_y")
                ps_y = state["ps_y"]
                for i in range(lo, hi):
                    jj, is_lo = divmod(i, 2)
                    if jj == JJ - 1:
                        # last pair swapped so the stop chain is full-height
                        # V_hi — the denominator rows' last writer must be
                        # the group-closing instruction, else the norm reads
                        # mid-accumulation-group
                        is_lo = 1 - is_lo
                    rhs = lst[jj][:, :, ha * CHUNK : (ha + 1) * CHUNK]
                    if is_lo:
                        nc.tensor.matmul(
                            ps_y[0:D, :],
                            lhsT=Vlo_sb[:, 2 * jj : 2 * jj + 2, h, :],
                            rhs=rhs,
                            start=False,
                            stop=(i == N_PVCH - 1),
                            perf_mode=DR,
                        )
                    else:
                        nc.tensor.matmul(
                            ps_y,
                            lhsT=Vhi_sb[:, 2 * jj : 2 * jj + 2, h, :],
                            rhs=rhs,
                            start=(i == 0),
                            stop=(i == N_PVCH - 1),
                            perf_mode=DR,
                        )
                if hi == N_PVCH:
                    if ha == 1:
                        exps.pop((ch, hp))
                    recip = norm_pool.tile([D, CHUNK], F32, tag="r", name="recip")
                    nc.vector.reciprocal(out=recip, in_=ps_y[D : 2 * D, :])
                    nc.vector.tensor_mul(
                        out=YT_sb[ha * D : (ha + 1) * D, hp, t0 : t0 + CHUNK],
                        in0=ps_y[0:D, :],
                        in1=recip,
                    )

            PV_PARTS = 4

            def emit_pv_ha(ch, hp, ha):
                state = {}
                for p in range(PV_PARTS):
                    emit_pv_chains(ch, hp, ha, N_PVCH * p // PV_PARTS,
                                   N_PVCH * (p + 1) // PV_PARTS, state)

            def pv_split(ch, hp, ha, cut):
                """Closures for chains [0, cut) in steps of 4, plus a tail fn
                emitting [cut, N_PVCH) + drain; all share one accumulator."""
                state = {}
                fills = [
                    (lambda lo=lo: emit_pv_chains(
                        ch, hp, ha, lo, min(lo + 4, cut), state))
                    for lo in range(0, cut, 4)
                ]
                def tail():
                    emit_pv_chains(ch, hp, ha, cut, N_PVCH, state)
                return fills, tail

            def emit_outproj_step(ch, mt, n2, kk, state, last=False):
                m = ch * (CHUNK // P) + mt
                if kk == 0 and n2 == 0:
                    state["o_sb"] = out_pool.tile([P, C], BF16, tag="o", name="o_sb")
                if kk == 0:
                    state[n2] = accps.tile([P, CHUNK], F32, tag="acc", name="ps_o")
                ps_o = state[n2]
                nc.tensor.matmul(
                    ps_o,
                    lhsT=YT_sb[:, kk, m * P : (m + 1) * P],
                    rhs=wpT_sb[:, kk, n2 * CHUNK : (n2 + 1) * CHUNK],
                    start=(kk == 0),
                    stop=(kk == KP - 1),
                )
                if kk == KP - 1:
                    o_sb = state["o_sb"]
                    dst = o_sb[:, n2 * CHUNK : (n2 + 1) * CHUNK]
                    # in the tail the exp stream is done: put half the drain
                    # copies on the freed ACT
                    if last and n2 % 2 == 0:
                        nc.scalar.copy(out=dst, in_=ps_o)
                    else:
                        nc.vector.tensor_copy(out=dst, in_=ps_o)
                    if n2 == C // CHUNK - 1:
                        eng = nc.sync if m % 2 == 0 else nc.gpsimd
                        eng.dma_start(
                            out=out_d[m * P : (m + 1) * P, :],
                            in_=o_sb,
                        )

            def emit_outproj_m(ch, mt, last=False):
                state = {}
                for n2 in range(C // CHUNK):
                    for kk in range(KP):
                        emit_outproj_step(ch, mt, n2, kk, state, last)

            # ---------- emission order ----------
            # Prep: only chunk-0 K and Q (copies on the still-idle ACT) so
            # the score stream starts as soon as x(ch0) lands; every other
            # projection, PV and outproj rides the stream as fillers with
            # deadlines encoded by position.
            emit_qk_group("k", 0, 0, on_act=True)
            emit_qk_group("q", 0, 0, on_act=True)

            def qk(which, j, ch):
                state = {}
                return [
                    (lambda ci=ci: emit_qk_chain(which, j, ch, ci, state, False))
                    for ci in range(3)
                ]

            def vg(m):
                state = {}
                return [
                    (lambda ci=ci: emit_v_chain(m, ci, state))
                    for ci in range(3)
                ]

            def pv(ch, hp, ha):
                state = {}
                return [
                    (lambda pt=pt: emit_pv_chains(
                        ch, hp, ha, N_PVCH * pt // PV_PARTS,
                        N_PVCH * (pt + 1) // PV_PARTS, state))
                    for pt in range(PV_PARTS)
                ]

            def op(ch, mt):
                state = {}
                return [
                    (lambda n2=n2, kk=kk: emit_outproj_step(ch, mt, n2, kk, state))
                    for n2 in range(2) for kk in range(KP)
                ]

            def flat(*fillers):
                return [f for sub in fillers for f in sub]

            # NOTE: Tile dependencies follow program order — every filler
            # must be emitted AFTER the writes it reads (a pv group needs
            # ALL 16 vg drains; scores of unit (ch,hp) need q(hp,ch) and
            # every k(hp,*) emitted in earlier positions).
            emit_sexp(0, 0, flat(
                qk("k", 0, 1), qk("k", 0, 2), qk("k", 0, 3),
                qk("k", 1, 0), qk("q", 1, 0),
            ))
            emit_sexp(0, 1, flat(
                qk("k", 1, 1), qk("k", 1, 2), qk("k", 1, 3),
                qk("q", 0, 1), vg(0), vg(1), vg(2), vg(3), vg(4),
            ))
            emit_sexp(1, 0, flat(
                qk("q", 1, 1), vg(5), vg(6), vg(7), vg(8), vg(9),
                vg(10),
            ))
            emit_sexp(1, 1, flat(
                qk("q", 0, 2), vg(11), vg(12), vg(13), vg(14), vg(15),
                pv(0, 0, 0), pv(0, 0, 1),
            ))
            emit_sexp(2, 0, flat(
                qk("q", 1, 2), pv(0, 1, 0), pv(0, 1, 1), pv(1, 0, 0),
            ))
            emit_sexp(2, 1, flat(
                qk("q", 0, 3), pv(1, 0, 1), pv(1, 1, 0), pv(1, 1, 1),
                op(0, 0), op(0, 1), op(0, 2),
            ))
            emit_sexp(3, 0, flat(
                qk("q", 1, 3), pv(2, 0, 0), pv(2, 0, 1),
                op(0, 3), op(1, 0), op(1, 1), op(1, 2),
            ))
            pv310_fills, pv310_tail = pv_split(3, 1, 0, 12)
            pv311_fills, pv311_tail = pv_split(3, 1, 1, 12)
            emit_sexp(3, 1, flat(
                pv(2, 1, 0), pv(2, 1, 1), pv(3, 0, 0), pv(3, 0, 1),
                op(1, 3), op(2, 0), pv310_fills, pv311_fills,
            ))
            # tail: pv completions first (they gate op(3) via the norms),
            # then ch2's remaining out tiles fill the PE while norms drain
            pv310_tail()
            pv311_tail()
            emit_outproj_m(2, 1, last=True)
            emit_outproj_m(2, 2, last=True)
            emit_outproj_m(2, 3, last=True)
            for mt in range(CHUNK // P):
                emit_outproj_m(3, mt, last=True)
    nc.finalize()
    return nc


def shard_inputs(x, Wk, Wq, Wv, Wp, T=2048):
    """Build the 8 per-core input dicts (hi/lo fp8 splits, host scaling)."""
    x = np.asarray(x, np.float32)
    Wk = np.asarray(Wk, np.float32)
    Wq = np.asarray(Wq, np.float32)
    Wv = np.asarray(Wv, np.float32)
    Wp = np.asarray(Wp, np.float32)

    def split8(a):
        hi = a.astype(NP_F8)
        lo = (a - hi.astype(np.float32)).astype(NP_F8)
        return hi, lo

    def pack(w):  # [C, DL] -> SBUF layout [P, KO*DL]
        KO = C // P
        return np.ascontiguousarray(
            w.reshape(KO, P, DL).transpose(1, 0, 2).reshape(P, KO * DL)
        )

    xs = []
    for b in range(x.shape[0]):
        xT = np.ascontiguousarray(x[b, :T].T)
        hi, lo = split8(xT)
        xs.append((np.ascontiguousarray(hi), np.ascontiguousarray(lo)))

    in_maps = []
    for g in range(N_GROUPS):
        sl = slice(g * DL, (g + 1) * DL)
        m = {}
        for n, W in (("wq", Wq), ("wk", Wk), ("wv", Wv)):
            hi, lo = split8(np.ascontiguousarray(W[sl].T * WSCALE))
            m[n + "h"] = pack(hi)
            m[n + "l"] = pack(lo)
        wp = (Wp[:, sl].T / WSCALE).astype(np.float32)  # [DL, C]
        KP = DL // P
        m["wpT"] = np.ascontiguousarray(
            wp.reshape(KP, P, C).transpose(1, 0, 2).reshape(P, KP * C)
        )
        for b in range(len(xs)):
            im = dict(m)
            im["xhi"], im["xlo"] = xs[b]
            in_maps.append(im)
    return in_maps


_PROGRAM = None


def kernel(x, Wk, Wq, Wv, Wp, bp):
    global _PROGRAM
    x = np.asarray(x, np.float32)
    bp = np.asarray(bp, np.float32)
    B, T, _ = x.shape

    if _PROGRAM is None:
        _PROGRAM = build_program(T)
    nc = _PROGRAM

    in_maps = shard_inputs(x, Wk, Wq, Wv, Wp, T=T)
    res = run_bass_kernel_spmd(nc, in_maps, core_ids=list(range(N_CORES)))
    parts = [r["out"] for r in res.results]

    out = np.zeros((B, T, C), np.float32)
    for g in range(N_GROUPS):
        for b in range(B):
            out[b] += parts[g * N_BATCH + b].astype(np.float32)
    out += bp
    return out


# revision 51
# speedup vs baseline: 1.0035x; 1.0022x over previous
"""Trainium2 Bass kernel for CausalSelfAttention (no causal mask in reference).

Problem shapes: x [B=2, T=2048, C=1024], H=16 heads, D=64 head dim.
  q/k/v = x @ W{q,k,v}.T ; att = softmax(q k^T / sqrt(D)) ; y = att v
  out = y @ Wp.T + bp

Sharding over 8 NeuronCores: 4 head-groups (4 heads = 256 dims each) x 2
batches.  Core (g, b) computes a partial output for x[b] restricted to head
group g; the host sums the 4 head-group partials per batch and adds bp.

v5 — hybrid fp8 DoubleRow scores + engine rebalance (134.3us, was
150.2us; rel_absmax 0.0174 of the 0.02 budget):
* QKV projections: hi/lo-COMPENSATED fp8 DoubleRow (x = x_hi + x_lo,
  16W = w_hi + w_lo; q = x_hi w_hi + x_hi w_lo + x_lo w_hi) — bf16-class
  accuracy at half the bf16 PE cost.  Weights are pre-scaled by 16 so the
  lo residuals stay in e4m3's normal range.
* Scores, hybrid by head-pair (the out-projection mixes heads, so fp8
  noise on half the heads enters the max-error metric at sqrt(1/2)):
  - hp=0: fp16 q/k, plain matmuls (error-free scores, 1 cyc/row);
  - hp=1: fp8e4 DoubleRow at 0.5 cyc/row with the DR ko slots carrying
    K_hi and K_lo (compensated K, zero extra matmuls) while q8 rides both
    slots via a stride-0 broadcast AP — S = (K_hi+K_lo)^T q8.  Only q's
    fp8 quantization adds error.  Cuts PE scores 131072 -> 98304 cycles.
* exp: ACT native Exp (fp8 out) for 88 tiles + 40 DVE tiles via the
  one-op Schraudolph trick: round(S*scale*8/ln2 + 55.62) written as int8
  IS the fp8e4 bit pattern of exp.  (GPSIMD cannot read PSUM on real
  TRN2 — BIR verifier — so it only runs DMAs and memsets.)
* PV: P in fp8e4 x V hi/lo-compensated DoubleRow, chains jj-grouped so
  the last unit can interleave PV with its own exp stream; ones columns
  in V_hi emit the softmax denominator on PSUM rows 64:128 (V_lo carries
  no aug columns and only accumulates rows 0:64).  The final V_hi/V_lo
  pair is swapped so the group-closing stop lands on a full-height pass.
* Output projection in f32r (same PE cost as bf16, fp32-accurate).
  Partials summed on host in fp32 (+bp); DMA'd out as bf16.
* Tile deps follow program order: every filler is emitted after the
  writes it reads (k/q before consumer units, all 16 vg before any pv,
  pv norms before op).
"""

import numpy as np
import ml_dtypes

import concourse.bass as bass
import concourse.tile as tile
from concourse import mybir
from concourse.bacc import Bacc
from concourse.bass_utils import run_bass_kernel_spmd

BF16 = mybir.dt.bfloat16
FP16 = mybir.dt.float16
F32 = mybir.dt.float32
F32R = mybir.dt.float32r
F8 = mybir.dt.float8e4
I8 = mybir.dt.int8
NP_BF16 = ml_dtypes.bfloat16
NP_F8 = mybir.dt.np(F8)

P = 128
C = 1024
H = 16
D = 64
N_CORES = 8
N_GROUPS = 4              # head groups (tensor parallel)
N_BATCH = 2               # data parallel over B
HL = H // N_GROUPS        # 4 local heads
DL = HL * D               # 256 local head dims
CHUNK = 512               # t-chunk width (one PSUM bank of fp32)

DR = mybir.MatmulPerfMode.DoubleRow
EXP = mybir.ActivationFunctionType.Exp

WSCALE = 16.0             # host pre-scale on Wq/Wk/Wv (see module docstring)
ESCALE = 0.125 / (WSCALE * WSCALE)
SCH_A = ESCALE * 8.0 / np.log(2.0)
SCH_B = 55.62

# exp slot -> engine per unit (unit = 2*ch + hp).  Default engine is
# ACT (native Exp); listed slots run the Schraudolph tensor_scalar on
# DVE instead.  More DVE slots = shorter ACT chain but more Schraudolph
# error; 40 slots measured 134284ns @ rel_absmax 0.0174.  (GPSIMD cannot
# read PSUM on real TRN2 — BIR verifier NCC_IBVF — so exp tiles can only
# run on ACT or DVE.)
DVE_SLOTS = (
    (2, 5, 8, 11, 14),
    (2, 5, 8, 11, 14),
    (2, 5, 8, 11, 14),
    (2, 5, 8, 11, 14),
    (2, 5, 8, 11, 14),
    (2, 5, 8, 11, 14),
    (2, 5, 8, 11, 14),
    (2, 4, 7, 9, 12, 14),
)


def build_program(T: int = 2048) -> bass.Bass:
    KO = C // P            # 8 k-tiles over the C contraction
    KPAIR = KO // 2        # 4 DoubleRow k-pairs
    TT = T // P            # 16 s/t tiles of 128
    NCH = T // CHUNK       # 4 t-chunks
    KP = DL // P           # 2 k-tiles over local head dims (outproj)

    nc = Bacc()
    xhi_d = nc.declare_dram_parameter("xhi", [C, T], F8, isOutput=False)
    xlo_d = nc.declare_dram_parameter("xlo", [C, T], F8, isOutput=False)
    # qkv weights arrive host-packed in SBUF layout [P, KO*DL] (2KB rows:
    # 256B rows would pay the sub-512B DMA descriptor penalty)
    w_d = {
        n: nc.declare_dram_parameter(n, [P, KO * DL], F8, isOutput=False)
        for n in ("wqh", "wql", "wkh", "wkl", "wvh", "wvl")
    }
    wpT_d = nc.declare_dram_parameter("wpT", [P, KP * C], F32R, isOutput=False)
    out_d = nc.declare_dram_parameter("out", [T, C], BF16, isOutput=True)

    with tile.TileContext(nc) as tc:
        with (
            tc.tile_pool(name="const", bufs=1) as cp,
            tc.tile_pool(name="att_s", bufs=2, space="PSUM") as att_s,
            tc.tile_pool(name="accy", bufs=2, space="PSUM") as accy,
            tc.tile_pool(name="accps", bufs=2, space="PSUM") as accps,
            tc.tile_pool(name="expp", bufs=38) as exp_pool,
            tc.tile_pool(name="normp", bufs=6) as norm_pool,
            tc.tile_pool(name="outp", bufs=4) as out_pool,
        ):
            xhi_sb = cp.tile([P, KO, T], F8)
            xlo_sb = cp.tile([P, KO, T], F8)
            w_sb = {n: cp.tile([P, KO, DL], F8, name=n) for n in w_d}
            wpT_sb = cp.tile([P, KP, C], F32R)
            # hybrid scores: head-pair hp=0 keeps bf16 q/k (error-free
            # scores); hp=1 uses fp8 q + hi/lo-compensated k in the DR ko
            # slots.  The out-projection mixes all heads, so the fp8 error
            # only enters at sqrt(1/2) weight.
            QTb_sb = cp.tile([P, T], FP16)
            KTb_sb = cp.tile([P, T], FP16)
            QT_sb = cp.tile([P, T], F8)
            KT_sb = cp.tile([P, 2, T], F8)
            # per (s-tile, head): V_hi = 64 V columns + 64 ones columns
            # (denominator rows); V_lo = 64 V columns only
            Vhi_sb = cp.tile([P, TT, HL, P], F8)
            Vlo_sb = cp.tile([P, TT, HL, D], F8)
            YT_sb = cp.tile([P, KP, T], F32R)

            # dummy matmuls on a memset tile keep the PE busy through the
            # initial DMA wait so the clock ramp completes before the first
            # real projection chain
            warm_sb = cp.tile([P, CHUNK], BF16)
            nc.vector.memset(warm_sb, 0.0)
            for _w in range(6):
                ps_w = accps.tile([P, CHUNK], F32, tag="acc", name="ps_w")
                nc.tensor.matmul(
                    ps_w, lhsT=warm_sb[:, 0:P], rhs=warm_sb, start=True, stop=True
                )

            # DMAs ordered by first use.  One DMA per (tensor, chunk) for x
            # (4KB rows), one per weight tensor: stays over the 500ns
            # descriptor-gen floor.
            def w_dma(eng, name):
                eng.dma_start(
                    out=w_sb[name][:, :, :],
                    in_=w_d[name][:, :].rearrange("p (ko d) -> p ko d", d=DL),
                )

            xhi_r = xhi_d[:, :].rearrange("(ko p) t -> p ko t", p=P)
            xlo_r = xlo_d[:, :].rearrange("(ko p) t -> p ko t", p=P)

            def x_dma(eng, sb, r, ch):
                sl = slice(ch * CHUNK, (ch + 1) * CHUNK)
                eng.dma_start(out=sb[:, :, sl], in_=r[:, :, sl])

            # Pool queue: k weights, x-lo ch0, q weights (prep deps first)
            w_dma(nc.gpsimd, "wkh")
            w_dma(nc.gpsimd, "wkl")
            x_dma(nc.gpsimd, xlo_sb, xlo_r, 0)
            w_dma(nc.gpsimd, "wqh")
            w_dma(nc.gpsimd, "wql")
            # SP queue: x-hi chunks, v weights, wp
            x_dma(nc.sync, xhi_sb, xhi_r, 0)
            x_dma(nc.sync, xhi_sb, xhi_r, 1)
            w_dma(nc.sync, "wvh")
            w_dma(nc.sync, "wvl")
            x_dma(nc.sync, xhi_sb, xhi_r, 2)
            x_dma(nc.sync, xhi_sb, xhi_r, 3)
            nc.sync.dma_start(
                out=wpT_sb[:, :, :],
                in_=wpT_d[:, :].rearrange("p (kp n) -> p kp n", n=C),
            )

            # V_hi aug ones (denominator weights), split so the Pool engine
            # stream can interleave its first exp tiles
            nc.gpsimd.memset(Vhi_sb[:, 0:8, :, D : 2 * D], 1.0)
            x_dma(nc.gpsimd, xlo_sb, xlo_r, 1)
            nc.gpsimd.memset(Vhi_sb[:, 8:TT, :, D : 2 * D], 1.0)
            x_dma(nc.gpsimd, xlo_sb, xlo_r, 2)
            x_dma(nc.gpsimd, xlo_sb, xlo_r, 3)

            # ---------- emission helpers ----------
            CHAINS = {  # compensated product: hi*hi + hi*lo + lo*hi
                "q": [(xhi_sb, "wqh"), (xhi_sb, "wql"), (xlo_sb, "wqh")],
                "k": [(xhi_sb, "wkh"), (xhi_sb, "wkl"), (xlo_sb, "wkh")],
                "v": [(xhi_sb, "wvh"), (xhi_sb, "wvl"), (xlo_sb, "wvh")],
            }

            def emit_qk_chain(which, j, ch, ci, state, on_act):
                # one compensation chain (4 DoubleRow matmuls); chain 0
                # allocates the accumulator, chain 2 drains it
                if ci == 0:
                    state["ps"] = accps.tile([P, CHUNK], F32, tag="acc", name="ps")
                ps = state["ps"]
                xs, wn = CHAINS[which][ci]
                for kk in range(KPAIR):
                    nc.tensor.matmul(
                        ps,
                        lhsT=w_sb[wn][:, 2 * kk : 2 * kk + 2, j * P : (j + 1) * P],
                        rhs=xs[:, 2 * kk : 2 * kk + 2, ch * CHUNK : (ch + 1) * CHUNK],
                        start=(ci == 0 and kk == 0),
                        stop=(ci == 2 and kk == KPAIR - 1),
                        perf_mode=DR,
                    )
                if ci == 2:
                    sl = slice(ch * CHUNK, (ch + 1) * CHUNK)
                    if j == 0:
                        dst = (QTb_sb if which == "q" else KTb_sb)[:, sl]
                        if on_act:
                            nc.scalar.copy(out=dst, in_=ps)
                        else:
                            nc.vector.tensor_copy(out=dst, in_=ps)
                    elif which == "q":
                        dst = QT_sb[:, sl]
                        if on_act:
                            nc.scalar.copy(out=dst, in_=ps)
                        else:
                            nc.vector.tensor_copy(out=dst, in_=ps)
                    else:
                        hi = KT_sb[:, 0, sl]
                        lo = KT_sb[:, 1, sl]
                        if on_act:
                            nc.scalar.copy(out=hi, in_=ps)
                        else:
                            nc.vector.tensor_copy(out=hi, in_=ps)
                        nc.vector.tensor_tensor(
                            out=lo, in0=ps, in1=hi,
                            op=mybir.AluOpType.subtract,
                        )

            def emit_qk_group(which, j, ch, on_act=False):
                state = {}
                for ci in range(3):
                    emit_qk_chain(which, j, ch, ci, state, on_act)

            def emit_v_chain(m, ci, state):
                if ci == 0:
                    state["ps"] = accps.tile([P, CHUNK], F32, tag="acc", name="ps")
                ps = state["ps"]
                xs, wn = CHAINS["v"][ci]
                for kk in range(KPAIR):
                    nc.tensor.matmul(
                        ps[:, 0:DL],
                        lhsT=xs[:, 2 * kk : 2 * kk + 2, m * P : (m + 1) * P],
                        rhs=w_sb[wn][:, 2 * kk : 2 * kk + 2, :],
                        start=(ci == 0 and kk == 0),
                        stop=(ci == 2 and kk == KPAIR - 1),
                        perf_mode=DR,
                    )
                if ci == 2:
                    vin = ps[:, 0:DL].rearrange("p (h e) -> p h e", e=D)
                    nc.vector.tensor_copy(out=Vhi_sb[:, m, :, 0:D], in_=vin)
                    nc.vector.tensor_tensor(
                        out=Vlo_sb[:, m, :, :],
                        in0=vin,
                        in1=Vhi_sb[:, m, :, 0:D],
                        op=mybir.AluOpType.subtract,
                    )

            exps = {}  # (ch, hp) -> list of 8 E tiles [P, 2, 2*CHUNK] fp8

            def emit_sexp(ch, hp, fillers=()):
                # score+exp stream for one (chunk, head-pair) unit; fillers
                # are PE work closures sprinkled between s-tiles so the PE
                # queue never head-of-line-blocks the exp engines
                t0 = ch * CHUNK
                u = 2 * ch + hp
                lst = []
                exps[(ch, hp)] = lst  # grows as tiles are created
                nf = len(fillers)
                fi = 0
                dve_s = DVE_SLOTS[u]
                for s in range(TT):
                    ps_s = att_s.tile([P, 2 * CHUNK], F32, tag="s", name="ps_s")
                    for ha in range(2):
                        dsl = slice(ha * D, (ha + 1) * D)
                        if hp == 0:
                            nc.tensor.matmul(
                                ps_s[:, ha * CHUNK : (ha + 1) * CHUNK],
                                lhsT=KTb_sb[dsl, s * P : (s + 1) * P],
                                rhs=QTb_sb[dsl, t0 : t0 + CHUNK],
                                start=True,
                                stop=True,
                            )
                        else:
                            nc.tensor.matmul(
                                ps_s[:, ha * CHUNK : (ha + 1) * CHUNK],
                                lhsT=KT_sb[dsl, :, s * P : (s + 1) * P],
                                rhs=QT_sb[dsl, t0 : t0 + CHUNK]
                                .unsqueeze(1)
                                .broadcast_to((D, 2, CHUNK)),
                                start=True,
                                stop=True,
                                perf_mode=DR,
                            )
                    if s % 2 == 0:
                        E = exp_pool.tile([P, 2, 2 * CHUNK], F8, tag="e", name="E")
                        lst.append(E)
                    dst = lst[-1][:, s % 2, :]
                    if s in dve_s:
                        nc.vector.tensor_scalar(
                            out=dst.bitcast(I8),
                            in0=ps_s,
                            scalar1=SCH_A,
                            scalar2=SCH_B,
                            op0=mybir.AluOpType.mult,
                            op1=mybir.AluOpType.add,
                        )
                    else:
                        nc.scalar.activation(out=dst, in_=ps_s, func=EXP, scale=ESCALE)
                    while fi < nf and fi < (s + 1) * nf // TT:
                        fillers[fi]()
                        fi += 1
                while fi < nf:
                    fillers[fi]()
                    fi += 1

            JJ = TT // 2
            N_PVCH = 2 * JJ

            def emit_pv_chains(ch, hp, ha, lo, hi, state):
                # jj-grouped chains: (V_hi, jj) then (V_lo, jj).  V_hi is
                # full-height (V + denominator aug); V_lo accumulates only
                # rows 0:64.  First chain (V_hi, 0) carries start (zeroes
                # the whole bank), last (V_lo, JJ-1) carries stop; the
                # normalization drain follows the last chain.
                t0 = ch * CHUNK
                lst = exps[(ch, hp)]
                h = hp * 2 + ha
                if lo == 0:
                    state["ps_y"] = accy.tile([P, CHUNK], F32, tag="y", name="ps_y")
                ps_y = state["ps_y"]
                for i in range(lo, hi):
                    jj, is_lo = divmod(i, 2)
                    if jj == JJ - 1:
                        # last pair swapped so the stop chain is full-height
                        # V_hi — the denominator rows' last writer must be
                        # the group-closing instruction, else the norm reads
                        # mid-accumulation-group
                        is_lo = 1 - is_lo
                    rhs = lst[jj][:, :, ha * CHUNK : (ha + 1) * CHUNK]
                    if is_lo:
                        nc.tensor.matmul(
                            ps_y[0:D, :],
                            lhsT=Vlo_sb[:, 2 * jj : 2 * jj + 2, h, :],
                            rhs=rhs,
                            start=False,
                            stop=(i == N_PVCH - 1),
                            perf_mode=DR,
                        )
                    else:
                        nc.tensor.matmul(
                            ps_y,
                            lhsT=Vhi_sb[:, 2 * jj : 2 * jj + 2, h, :],
                            rhs=rhs,
                            start=(i == 0),
                            stop=(i == N_PVCH - 1),
                            perf_mode=DR,
                        )
                if hi == N_PVCH:
                    if ha == 1:
                        exps.pop((ch, hp))
                    recip = norm_pool.tile([D, CHUNK], F32, tag="r", name="recip")
                    nc.vector.reciprocal(out=recip, in_=ps_y[D : 2 * D, :])
                    nc.vector.tensor_mul(
                        out=YT_sb[ha * D : (ha + 1) * D, hp, t0 : t0 + CHUNK],
                        in0=ps_y[0:D, :],
                        in1=recip,
                    )

            PV_PARTS = 4

            def emit_pv_ha(ch, hp, ha):
                state = {}
                for p in range(PV_PARTS):
                    emit_pv_chains(ch, hp, ha, N_PVCH * p // PV_PARTS,
                                   N_PVCH * (p + 1) // PV_PARTS, state)

            def pv_split(ch, hp, ha, cut):
                """Closures for chains [0, cut) in steps of 4, plus a tail fn
                emitting [cut, N_PVCH) + drain; all share one accumulator."""
                state = {}
                fills = [
                    (lambda lo=lo: emit_pv_chains(
                        ch, hp, ha, lo, min(lo + 4, cut), state))
                    for lo in range(0, cut, 4)
                ]
                def tail():
                    emit_pv_chains(ch, hp, ha, cut, N_PVCH, state)
                return fills, tail

            def emit_outproj_step(ch, mt, n2, kk, state, last=False):
                m = ch * (CHUNK // P) + mt
                if kk == 0 and n2 == 0:
                    state["o_sb"] = out_pool.tile([P, C], BF16, tag="o", name="o_sb")
                if kk == 0:
                    state[n2] = accps.tile([P, CHUNK], F32, tag="acc", name="ps_o")
                ps_o = state[n2]
                nc.tensor.matmul(
                    ps_o,
                    lhsT=YT_sb[:, kk, m * P : (m + 1) * P],
                    rhs=wpT_sb[:, kk, n2 * CHUNK : (n2 + 1) * CHUNK],
                    start=(kk == 0),
                    stop=(kk == KP - 1),
                )
                if kk == KP - 1:
                    o_sb = state["o_sb"]
                    dst = o_sb[:, n2 * CHUNK : (n2 + 1) * CHUNK]
                    # in the tail the exp stream is done: put half the drain
                    # copies on the freed ACT
                    if last and n2 % 2 == 0:
                        nc.scalar.copy(out=dst, in_=ps_o)
                    else:
                        nc.vector.tensor_copy(out=dst, in_=ps_o)
                    if last:
                        # tail: per-half DMAs so the final transfer is
                        # smaller and starts right after its own drain
                        eng = nc.sync if n2 % 2 == 0 else nc.gpsimd
                        eng.dma_start(
                            out=out_d[m * P : (m + 1) * P,
                                      n2 * CHUNK : (n2 + 1) * CHUNK],
                            in_=dst,
                        )
                    elif n2 == C // CHUNK - 1:
                        eng = nc.sync if m % 2 == 0 else nc.gpsimd
                        eng.dma_start(
                            out=out_d[m * P : (m + 1) * P, :],
                            in_=o_sb,
                        )

            def emit_outproj_m(ch, mt, last=False):
                state = {}
                for n2 in range(C // CHUNK):
                    for kk in range(KP):
                        emit_outproj_step(ch, mt, n2, kk, state, last)

            # ---------- emission order ----------
            # Prep: only chunk-0 K and Q (copies on the still-idle ACT) so
            # the score stream starts as soon as x(ch0) lands; every other
            # projection, PV and outproj rides the stream as fillers with
            # deadlines encoded by position.
            emit_qk_group("k", 0, 0, on_act=True)
            emit_qk_group("q", 0, 0, on_act=True)

            def qk(which, j, ch):
                state = {}
                return [
                    (lambda ci=ci: emit_qk_chain(which, j, ch, ci, state, False))
                    for ci in range(3)
                ]

            def vg(m):
                state = {}
                return [
                    (lambda ci=ci: emit_v_chain(m, ci, state))
                    for ci in range(3)
                ]

            def pv(ch, hp, ha):
                state = {}
                return [
                    (lambda pt=pt: emit_pv_chains(
                        ch, hp, ha, N_PVCH * pt // PV_PARTS,
                        N_PVCH * (pt + 1) // PV_PARTS, state))
                    for pt in range(PV_PARTS)
                ]

            def op(ch, mt):
                state = {}
                return [
                    (lambda n2=n2, kk=kk: emit_outproj_step(ch, mt, n2, kk, state))
                    for n2 in range(2) for kk in range(KP)
                ]

            def flat(*fillers):
                return [f for sub in fillers for f in sub]

            # NOTE: Tile dependencies follow program order — every filler
            # must be emitted AFTER the writes it reads (a pv group needs
            # ALL 16 vg drains; scores of unit (ch,hp) need q(hp,ch) and
            # every k(hp,*) emitted in earlier positions).
            emit_sexp(0, 0, flat(
                qk("k", 0, 1), qk("k", 0, 2), qk("k", 0, 3),
                qk("k", 1, 0), qk("q", 1, 0),
            ))
            emit_sexp(0, 1, flat(
                qk("k", 1, 1), qk("k", 1, 2), qk("k", 1, 3),
                qk("q", 0, 1), vg(0), vg(1), vg(2), vg(3), vg(4),
            ))
            emit_sexp(1, 0, flat(
                qk("q", 1, 1), vg(5), vg(6), vg(7), vg(8), vg(9),
                vg(10),
            ))
            emit_sexp(1, 1, flat(
                qk("q", 0, 2), vg(11), vg(12), vg(13), vg(14), vg(15),
                pv(0, 0, 0), pv(0, 0, 1),
            ))
            emit_sexp(2, 0, flat(
                qk("q", 1, 2), pv(0, 1, 0), pv(0, 1, 1), pv(1, 0, 0),
            ))
            emit_sexp(2, 1, flat(
                qk("q", 0, 3), pv(1, 0, 1), pv(1, 1, 0), pv(1, 1, 1),
                op(0, 0), op(0, 1), op(0, 2),
            ))
            emit_sexp(3, 0, flat(
                qk("q", 1, 3), pv(2, 0, 0), pv(2, 0, 1),
                op(0, 3), op(1, 0), op(1, 1), op(1, 2),
            ))
            pv310_fills, pv310_tail = pv_split(3, 1, 0, 14)
            pv311_fills, pv311_tail = pv_split(3, 1, 1, 14)
            emit_sexp(3, 1, flat(
                pv(2, 1, 0), pv(2, 1, 1), pv(3, 0, 0), pv(3, 0, 1),
                op(1, 3), op(2, 0), pv310_fills, pv311_fills,
            ))
            # tail: pv completions first (they gate op(3) via the norms),
            # then ch2's remaining out tiles fill the PE while norms drain
            pv310_tail()
            pv311_tail()
            emit_outproj_m(2, 1, last=True)
            emit_outproj_m(2, 2, last=True)
            emit_outproj_m(2, 3, last=True)
            for mt in range(CHUNK // P):
                emit_outproj_m(3, mt, last=True)
    nc.finalize()
    return nc


def shard_inputs(x, Wk, Wq, Wv, Wp, T=2048):
    """Build the 8 per-core input dicts (hi/lo fp8 splits, host scaling)."""
    x = np.asarray(x, np.float32)
    Wk = np.asarray(Wk, np.float32)
    Wq = np.asarray(Wq, np.float32)
    Wv = np.asarray(Wv, np.float32)
    Wp = np.asarray(Wp, np.float32)

    def split8(a):
        hi = a.astype(NP_F8)
        lo = (a - hi.astype(np.float32)).astype(NP_F8)
        return hi, lo

    def pack(w):  # [C, DL] -> SBUF layout [P, KO*DL]
        KO = C // P
        return np.ascontiguousarray(
            w.reshape(KO, P, DL).transpose(1, 0, 2).reshape(P, KO * DL)
        )

    xs = []
    for b in range(x.shape[0]):
        xT = np.ascontiguousarray(x[b, :T].T)
        hi, lo = split8(xT)
        xs.append((np.ascontiguousarray(hi), np.ascontiguousarray(lo)))

    in_maps = []
    for g in range(N_GROUPS):
        sl = slice(g * DL, (g + 1) * DL)
        m = {}
        for n, W in (("wq", Wq), ("wk", Wk), ("wv", Wv)):
            hi, lo = split8(np.ascontiguousarray(W[sl].T * WSCALE))
            m[n + "h"] = pack(hi)
            m[n + "l"] = pack(lo)
        wp = (Wp[:, sl].T / WSCALE).astype(np.float32)  # [DL, C]
        KP = DL // P
        m["wpT"] = np.ascontiguousarray(
            wp.reshape(KP, P, C).transpose(1, 0, 2).reshape(P, KP * C)
        )
        for b in range(len(xs)):
            im = dict(m)
            im["xhi"], im["xlo"] = xs[b]
            in_maps.append(im)
    return in_maps


_PROGRAM = None


def kernel(x, Wk, Wq, Wv, Wp, bp):
    global _PROGRAM
    x = np.asarray(x, np.float32)
    bp = np.asarray(bp, np.float32)
    B, T, _ = x.shape

    if _PROGRAM is None:
        _PROGRAM = build_program(T)
    nc = _PROGRAM

    in_maps = shard_inputs(x, Wk, Wq, Wv, Wp, T=T)
    res = run_bass_kernel_spmd(nc, in_maps, core_ids=list(range(N_CORES)))
    parts = [r["out"] for r in res.results]

    out = np.zeros((B, T, C), np.float32)
    for g in range(N_GROUPS):
        for b in range(B):
            out[b] += parts[g * N_BATCH + b].astype(np.float32)
    out += bp
    return out
